# revision 1
# baseline (speedup 1.0000x reference)
"""Trainium2 Bass kernel for nn_BuildModel_3796751089773.

RAIM-attention + LSTMCell scan over T=256 steps, B=1024, F=128, H=256, W=3,
followed by sum-over-time prediction head -> [B, 1].

Strategy (8 cores, data-parallel over batch, B_local = 128 = SBUF partitions):
  - Normal layout [batch_partitions, feature_free] for attention softmax and
    all elementwise work (per-partition scalars make softmax/z cheap).
  - gates = z @ W_ih^T + h @ W_hh^T computed with activations-transposed as
    PE stationary (hT/zT via PE transposes), weights streaming as rhs.
  - Output head sum_t(h_t) @ w_pred^T accumulated in a persistent PSUM bank
    by riding tiny N=1 matmuls on the already-loaded hT stationaries.
  - sigmoid(x) = 0.5*(1+tanh(x/2)) so the only ACT functions used are
    tanh/exp/copy -> one ACT table set ("exp_and_others"), loaded once.
  - h,c state kept doubled (H=2h, C=2c) so the LSTM update is exactly three
    fused scalar_tensor_tensor ops; the 0.5 factors fold into weights.
  - x-dependent parts of alpha/beta preacts fold into the SAME PSUM
    accumulation via 3 static rhs matrices against transposed x slices
    (no separate xa/xb precompute, no extra passes over v).
  - fp32 storage everywhere; matmuls run as float32r (full-rate for N>=256).
"""

import os
import sys

import numpy as np

for _p in ("/opt/trn_rl_repo",):
    if _p not in sys.path:
        sys.path.insert(0, _p)

import concourse.bass as bass
import concourse.bacc as bacc
import concourse.tile as tile
from concourse import mybir
from concourse.bass_utils import run_bass_kernel_spmd
from concourse.masks import make_identity
from concourse.dve_ops import (
    OPS as _DVE_OPS, CUSTOM_DVE_SPECS as _DVE_SPECS,
    _SUB_OPCODE_FOR_NAME as _DVE_ROWS, _CUSTOM_DVE_ROW_BASE as _DVE_ROW_BASE,
    DveOp as _DveOp,
)
from concourse.dve_spec import Spec as _Spec, Src0 as _Src0, Src1 as _Src1, \
    C0 as _C0, C1 as _C1, lower as _dve_lower
from concourse.dve_uop import DveOpSpec as _DveOpSpec


def _register_u2_op():
    """out = in0*s0 + in1*s1 with two per-partition scalars (one DVE inst)."""
    name = "U2_MULADD_ANT"
    if name in _DVE_ROWS:
        return next(o for o in _DVE_OPS if o.name == name)
    spec = _Spec(
        body=_Src0 * _C0 + _Src1 * _C1,
        reference=lambda in0, in1, s0, s1, imm2:
            in0.astype(np.float32) * s0 + in1.astype(np.float32) * s1,
    )
    row = _DVE_ROW_BASE + len(_DVE_OPS)
    _DVE_ROWS[name] = row
    shas = {}
    for ver in ("v3", "v4"):
        try:
            uops = _dve_lower(spec, ver=ver)
            shas[ver] = _DveOpSpec(name=name, opcode=row, uops=uops,
                                   rd1_en=True).sha(ver)
        except Exception:
            pass
    op = _DveOp(name, spec, subdim=False, uops_sha=shas)
    _DVE_OPS.append(op)
    _DVE_SPECS[name] = spec
    return op


U2_OP = _register_u2_op()

B, T, F, W, H, L = 1024, 256, 128, 3, 256, 1
NCORES = 8
BL = B // NCORES  # 128
AF = mybir.ActivationFunctionType
ALU = mybir.AluOpType
DT = mybir.dt
F32 = DT.float32
F32R = DT.float32r

_CACHE = {}


def build_kernel(gate_bias_nonzero, ab_bias_nonzero):
    nc = bacc.Bacc("TRN2")

    xn_d = nc.dram_tensor("xn", [T, BL, F], F32, kind="ExternalInput")
    xt_d = nc.dram_tensor("xt", [T, F, BL], F32R, kind="ExternalInput")
    wg_d = nc.dram_tensor("wg", [3, 128, 1024], F32R, kind="ExternalInput")
    wab_d = nc.dram_tensor("wab", [2, 128, 256], F32R, kind="ExternalInput")
    wr_d = nc.dram_tensor("wr", [3, 128, 256], F32R, kind="ExternalInput")
    wp_d = nc.dram_tensor("wp", [2, 128, 8], F32R, kind="ExternalInput")
    bg_d = nc.dram_tensor("bg", [1, 1024], F32R, kind="ExternalInput")
    bab_d = nc.dram_tensor("bab", [1, 256], F32R, kind="ExternalInput")
    y_d = nc.dram_tensor("y", [BL, L], F32, kind="ExternalOutput")

    from contextlib import ExitStack

    with tile.TileContext(nc) as tc, ExitStack() as ctx:
        singles = ctx.enter_context(tc.tile_pool(name="singles", bufs=1))
        xn_pool = ctx.enter_context(tc.tile_pool(name="xn", bufs=3))
        xt_pool = ctx.enter_context(tc.tile_pool(name="xt", bufs=3))
        work = ctx.enter_context(tc.tile_pool(name="work", bufs=2))
        ab_pool = ctx.enter_context(tc.tile_pool(name="abps", bufs=3, space="PSUM"))
        g_pool = ctx.enter_context(tc.tile_pool(name="gps", bufs=1, space="PSUM"))
        tr_pool = ctx.enter_context(tc.tile_pool(name="trps", bufs=2, space="PSUM"))
        y_pool = ctx.enter_context(tc.tile_pool(name="yps", bufs=1, space="PSUM"))

        # ---- one-time loads -------------------------------------------------
        wg_s = singles.tile([128, 3, 1024], F32R)
        wab_s = singles.tile([128, 2, 256], F32R)
        wr_s = singles.tile([128, 3, 256], F32R)
        wp_s = singles.tile([128, 2, 8], F32R)
        for k in range(3):
            nc.sync.dma_start(out=wg_s[:, k, :], in_=wg_d[k])
            nc.sync.dma_start(out=wr_s[:, k, :], in_=wr_d[k])
        for k in range(2):
            nc.sync.dma_start(out=wab_s[:, k, :], in_=wab_d[k])
            nc.sync.dma_start(out=wp_s[:, k, :], in_=wp_d[k])
        ident = singles.tile([128, 128], F32)
        make_identity(nc, ident)
        ones_row = None
        bg_s = bab_s = None
        if gate_bias_nonzero or ab_bias_nonzero:
            ones_row = singles.tile([1, 128], F32R)
            nc.vector.memset(ones_row, 1.0)
        if gate_bias_nonzero:
            bg_s = singles.tile([1, 1024], F32R)
            nc.sync.dma_start(out=bg_s, in_=bg_d[:])
        if ab_bias_nonzero:
            bab_s = singles.tile([1, 256], F32R)
            nc.sync.dma_start(out=bab_s, in_=bab_d[:])

        # ---- persistent state ----------------------------------------------
        H_s = singles.tile([128, 256], F32)  # doubled hidden state 2*h
        C_s = singles.tile([128, 256], F32)  # doubled cell state 2*c
        # same engine as make_identity so early consumers coalesce to one wait
        nc.gpsimd.memset(H_s, 0.0)
        nc.gpsimd.memset(C_s, 0.0)

        y_ps = y_pool.tile([128, 8], F32)

        ab_tiles = {}
        xn_tiles = {}

        def new_ab(tau):
            ab_tiles[tau] = ab_pool.tile([128, 256], F32, tag="ab",
                                         name=f"ab{tau}")
            if ab_bias_nonzero:
                nc.tensor.matmul(
                    ab_tiles[tau], ones_row, bab_s, start=True, stop=False
                )

        # ab PSUM start flag handling: first writer per tile.
        # tau >= 2: first writer is the R0 matmul issued at step tau-2.
        # tau == 1: first writer is the R1 matmul issued at step 0.
        # tau == 0: first writer is the R2 matmul issued at step 0.
        # When biases are nonzero the bias matmul above is the first writer.
        def ab_start(tau, first):
            return first and not ab_bias_nonzero

        GB = 4  # steps per DMA group
        xn_groups = {}
        xt_groups = {}
        for t in range(T):
            # -- DMA x slices in (batched, 4 steps per transfer) ---------------
            if t % GB == 0:
                gi = t // GB
                gxn = xn_pool.tile([128, GB, 128], F32, tag="xn",
                                   name=f"xng{gi}")
                nc.sync.dma_start(out=gxn,
                                  in_=xn_d[t:t + GB].rearrange("k p f -> p k f"))
                xn_groups[gi] = gxn
                gxt = xt_pool.tile([128, GB, 128], F32R, tag="xt",
                                   name=f"xtg{gi}")
                nc.sync.dma_start(out=gxt,
                                  in_=xt_d[t:t + GB].rearrange("k p f -> p k f"))
                xt_groups[gi] = gxt
            xn_tiles[t] = xn_groups[t // GB][:, t % GB, :]
            xt_t = xt_groups[t // GB][:, t % GB, :]

            # -- x contributions to alpha/beta preacts of steps t, t+1, t+2 ----
            if t == 0:
                for tau in range(min(3, T)):
                    new_ab(tau)
            elif t + 2 < T:
                new_ab(t + 2)
            # R2 -> psum[t], R1 -> psum[t+1], R0 -> psum[t+2]
            nc.tensor.matmul(
                ab_tiles[t], xt_t, wr_s[:, 2, :],
                start=ab_start(t, t == 0), stop=False,
            )
            if t + 1 < T:
                nc.tensor.matmul(
                    ab_tiles[t + 1], xt_t, wr_s[:, 1, :],
                    start=ab_start(t + 1, t == 0), stop=False,
                )
            if t + 2 < T:
                nc.tensor.matmul(
                    ab_tiles[t + 2], xt_t, wr_s[:, 0, :],
                    start=ab_start(t + 2, True), stop=False,
                )

            # -- transpose H (state from step t-1) ------------------------------
            tr_t = tr_pool.tile([128, 384], F32, tag="tr")
            nc.tensor.transpose(tr_t[:, 0:128], H_s[:, 0:128], ident)
            nc.tensor.transpose(tr_t[:, 128:256], H_s[:, 128:256], ident)
            hT01 = work.tile([128, 256], F32R, tag="hT01")
            nc.scalar.copy(out=hT01, in_=tr_t[:, 0:256])
            hT0 = hT01[:, 0:128]
            hT1 = hT01[:, 128:256]

            # -- h-dependent matmuls: alpha/beta + gates-h + ypred --------------
            g_ps = g_pool.tile([128, 1024], F32, tag="g")
            nc.tensor.matmul(ab_tiles[t], hT0, wab_s[:, 0, :],
                             start=False, stop=False)
            nc.tensor.matmul(g_ps[:, 0:512], hT0, wg_s[:, 1, 0:512],
                             start=not gate_bias_nonzero, stop=False)
            nc.tensor.matmul(g_ps[:, 512:1024], hT0, wg_s[:, 1, 512:1024],
                             start=not gate_bias_nonzero, stop=False)
            if gate_bias_nonzero and t >= 0:
                # bias first writer for this step's gate PSUM
                pass
            nc.tensor.matmul(y_ps, hT0, wp_s[:, 0, :],
                             start=(t == 0), stop=False)
            nc.tensor.matmul(ab_tiles[t], hT1, wab_s[:, 1, :],
                             start=False, stop=True)
            nc.tensor.matmul(g_ps[:, 0:512], hT1, wg_s[:, 2, 0:512],
                             start=False, stop=False)
            nc.tensor.matmul(g_ps[:, 512:1024], hT1, wg_s[:, 2, 512:1024],
                             start=False, stop=False)
            nc.tensor.matmul(y_ps, hT1, wp_s[:, 1, :],
                             start=False, stop=False)

            # -- attention softmax path ----------------------------------------
            t_ab = work.tile([128, 131], F32, tag="tab")
            nc.scalar.activation(out=t_ab, in_=ab_tiles[t][:, 0:131], func=AF.Tanh)
            e_a = work.tile([128, 3], F32, tag="ea")
            e_b = work.tile([128, 128], F32, tag="eb")
            s_a = work.tile([128, 1], F32, tag="sa")
            s_b = work.tile([128, 1], F32, tag="sb")
            nc.scalar.activation(out=e_a, in_=t_ab[:, 0:3], func=AF.Exp,
                                 accum_out=s_a)
            nc.scalar.activation(out=e_b, in_=t_ab[:, 3:131], func=AF.Exp,
                                 accum_out=s_b)
            s_ab = work.tile([128, 1], F32, tag="sab")
            nc.vector.tensor_mul(s_ab, s_a, s_b)
            r_ab = work.tile([128, 1], F32, tag="rab")
            nc.vector.reciprocal(r_ab, s_ab)

            # u = sum_w e_alpha[w] * x_{t-2+w}
            u = work.tile([128, 128], F32, tag="u")
            if t == 0:
                nc.vector.tensor_scalar_mul(u, xn_tiles[0], e_a[:, 2:3])
            elif t == 1:
                nc.vector._custom_dve(
                    U2_OP, out=u, in0=xn_tiles[0], in1=xn_tiles[1],
                    s0=e_a[:, 1:2], s1=e_a[:, 2:3])
            else:
                u01 = work.tile([128, 128], F32, tag="u01")
                nc.vector._custom_dve(
                    U2_OP, out=u01, in0=xn_tiles[t - 2], in1=xn_tiles[t - 1],
                    s0=e_a[:, 0:1], s1=e_a[:, 1:2])
                nc.vector.scalar_tensor_tensor(
                    out=u, in0=xn_tiles[t], scalar=e_a[:, 2:3], in1=u01,
                    op0=ALU.mult, op1=ALU.add)
            # z = e_beta * u * r  (normalized attention output)
            z = work.tile([128, 128], F32, tag="z")
            nc.vector.scalar_tensor_tensor(
                out=z, in0=u, scalar=r_ab, in1=e_b, op0=ALU.mult, op1=ALU.mult)

            # -- zT and gates-z -------------------------------------------------
            nc.tensor.transpose(tr_t[:, 256:384], z, ident)
            zT = work.tile([128, 128], F32R, tag="zT")
            nc.vector.tensor_copy(out=zT, in_=tr_t[:, 256:384])
            if gate_bias_nonzero:
                nc.tensor.matmul(g_ps[:, 0:512], ones_row, bg_s[:, 0:512],
                                 start=False, stop=False)
                nc.tensor.matmul(g_ps[:, 512:1024], ones_row,
                                 bg_s[:, 512:1024], start=False, stop=False)
            nc.tensor.matmul(g_ps[:, 0:512], zT, wg_s[:, 0, 0:512],
                             start=False, stop=True)
            nc.tensor.matmul(g_ps[:, 512:1024], zT, wg_s[:, 0, 512:1024],
                             start=False, stop=True)

            # -- gate activations: cols [i(256) f(256) o(256) g(256)] ----------
            # cols: f 0:256, i 256:512, o 512:768, g 768:1024
            tg4 = work.tile([128, 1024], F32, tag="tg4")
            nc.scalar.activation(out=tg4[:, 0:512], in_=g_ps[:, 0:512],
                                 func=AF.Tanh, scale=0.5)

            # -- LSTM state update (doubled state) ------------------------------
            # A = (1+tanh(f/2)) * C   (= 4*sig(f)*c)
            A_t = work.tile([128, 256], F32, tag="A")
            nc.vector.scalar_tensor_tensor(
                out=A_t, in0=tg4[:, 0:256], scalar=1.0, in1=C_s,
                op0=ALU.add, op1=ALU.mult)
            nc.scalar.activation(out=tg4[:, 768:1024], in_=g_ps[:, 768:1024],
                                 func=AF.Tanh)
            # Q = (1+tanh(i/2)) * tanh(g)   (= 2*sig(i)*tanh(g))
            Q_t = work.tile([128, 256], F32, tag="Q")
            nc.vector.scalar_tensor_tensor(
                out=Q_t, in0=tg4[:, 256:512], scalar=1.0, in1=tg4[:, 768:1024],
                op0=ALU.add, op1=ALU.mult)
            nc.scalar.activation(out=tg4[:, 512:768], in_=g_ps[:, 512:768],
                                 func=AF.Tanh, scale=0.5)
            # C_new = 0.5*A + Q   (= 2*c_new)
            nc.vector.scalar_tensor_tensor(
                out=C_s, in0=A_t, scalar=0.5, in1=Q_t,
                op0=ALU.mult, op1=ALU.add)
            # tanh(c_new) = tanh(0.5*C)
            t_c = work.tile([128, 256], F32, tag="tc")
            nc.scalar.activation(out=t_c, in_=C_s, func=AF.Tanh, scale=0.5)
            # H_new = (1+tanh(o/2)) * tanh(c_new)   (= 2*h_new)
            nc.vector.scalar_tensor_tensor(
                out=H_s, in0=tg4[:, 512:768], scalar=1.0, in1=t_c,
                op0=ALU.add, op1=ALU.mult)

        # ---- final h contribution to y + writeback --------------------------
        tr_f = tr_pool.tile([128, 384], F32, tag="tr")
        nc.tensor.transpose(tr_f[:, 0:128], H_s[:, 0:128], ident)
        nc.tensor.transpose(tr_f[:, 128:256], H_s[:, 128:256], ident)
        hT01f = work.tile([128, 256], F32R, tag="hT01")
        nc.scalar.copy(out=hT01f, in_=tr_f[:, 0:256])
        hT0f = hT01f[:, 0:128]
        hT1f = hT01f[:, 128:256]
        nc.tensor.matmul(y_ps, hT0f, wp_s[:, 0, :], start=False, stop=False)
        nc.tensor.matmul(y_ps, hT1f, wp_s[:, 1, :], start=False, stop=True)
        y_sb = work.tile([128, 1], F32, tag="ysb")
        nc.scalar.copy(out=y_sb, in_=y_ps[:, 0:1])
        nc.sync.dma_start(out=y_d[:], in_=y_sb)

    nc.finalize()
    return nc


def _prep_inputs(v, w_h_alpha, b_h_alpha, w_a_alpha, b_a_alpha,
                 w_h_beta, b_h_beta, w_a_beta, b_a_beta,
                 w_ih, b_ih, w_hh, b_hh, w_pred, b_pred):
    v = np.ascontiguousarray(np.asarray(v, dtype=np.float32))
    # gate row reorder: torch order (i,f,g,o) -> (i,f,o,g)
    idx = np.concatenate([np.arange(H, 2 * H), np.arange(0, H),
                          np.arange(3 * H, 4 * H), np.arange(2 * H, 3 * H)])
    wih_p = np.asarray(w_ih, np.float32)[idx]          # [1024, 128]
    whh_p = np.asarray(w_hh, np.float32)[idx]          # [1024, 256]
    bg = (np.asarray(b_ih, np.float32) + np.asarray(b_hh, np.float32))[idx]

    wg = np.zeros((3, 128, 1024), np.float32)
    wg[0] = wih_p.T
    wg[1] = 0.5 * whh_p.T[0:128]
    wg[2] = 0.5 * whh_p.T[128:256]

    wab = np.zeros((2, 128, 256), np.float32)
    wha_t = np.asarray(w_h_alpha, np.float32).T        # [H, 3]
    whb_t = np.asarray(w_h_beta, np.float32).T         # [H, F]
    for k in range(2):
        wab[k, :, 0:3] = 0.5 * wha_t[128 * k:128 * (k + 1)]
        wab[k, :, 3:131] = 0.5 * whb_t[128 * k:128 * (k + 1)]

    wr = np.zeros((3, 128, 256), np.float32)
    waa = np.asarray(w_a_alpha, np.float32)[0]         # [F]
    wab_beta = np.asarray(w_a_beta, np.float32)[0]     # [W]
    eye = np.eye(128, dtype=np.float32)
    for d in range(3):
        wr[d, :, d] = waa
        wr[d, :, 3:131] = wab_beta[d] * eye

    wp = np.zeros((2, 128, 8), np.float32)
    wp[:, :, 0] = (0.5 * np.asarray(w_pred, np.float32)[0]).reshape(2, 128)

    bab = np.zeros((1, 256), np.float32)
    bab[0, 0:3] = np.asarray(b_h_alpha, np.float32) + np.asarray(b_a_alpha,
                                                                 np.float32)[0]
    bab[0, 3:131] = np.asarray(b_h_beta, np.float32) + np.asarray(b_a_beta,
                                                                  np.float32)[0]

    gate_bias_nonzero = bool(np.any(bg != 0.0))
    ab_bias_nonzero = bool(np.any(bab != 0.0))

    shared = {
        "wg": wg, "wab": wab, "wr": wr, "wp": wp,
        "bg": bg.reshape(1, 1024), "bab": bab,
    }
    in_maps = []
    vs = v.reshape(NCORES, BL, T, F)
    for c in range(NCORES):
        vc = vs[c]                                     # [BL, T, F]
        in_maps.append({
            "xn": np.ascontiguousarray(vc.transpose(1, 0, 2)),  # [T, BL, F]
            "xt": np.ascontiguousarray(vc.transpose(1, 2, 0)),  # [T, F, BL]
            **shared,
        })
    b_pred_total = float(T) * np.asarray(b_pred, np.float32)    # [L]
    return in_maps, gate_bias_nonzero, ab_bias_nonzero, b_pred_total


def _run(inputs, trace=False):
    in_maps, gb_nz, ab_nz, b_pred_total = _prep_inputs(**inputs)
    key = (gb_nz, ab_nz)
    if key not in _CACHE:
        _CACHE[key] = build_kernel(gb_nz, ab_nz)
    nc = _CACHE[key]
    res = run_bass_kernel_spmd(
        nc, in_maps, core_ids=list(range(NCORES)), trace=trace,
    )
    y = np.concatenate([res.results[c]["y"] for c in range(NCORES)], axis=0)
    y = y + b_pred_total[None, :]
    return np.asarray(y, dtype=np.float32), res


def kernel(**inputs):
    y, _ = _run(inputs, trace=False)
    return y



# revision 5
# speedup vs baseline: 1.2929x; 1.2929x over previous
"""Trainium2 Bass kernel for nn_BuildModel_3796751089773.

RAIM-attention + LSTMCell scan over T=256 steps, B=1024, F=128, H=256, W=3,
followed by sum-over-time prediction head -> [B, 1].

Strategy (8 cores, data-parallel over batch, B_local = 128 = SBUF partitions):
  - Normal layout [batch_partitions, feature_free] for attention softmax and
    all elementwise work (per-partition scalars make softmax/z cheap).
  - All matmuls/transposes in bf16 (1 cycle/row at any N, cheap LDWEIGHTS);
    PSUM accumulation stays fp32.  Cell state C kept fp32, everything else
    that feeds matmuls is bf16.
  - gates = z @ W_ih^T + h @ W_hh^T with activations-transposed as PE
    stationary (hT/zT via PE transposes), weights streaming as rhs.
  - sigmoid(x) = 0.5*(1+tanh(x/2)) so the only ACT functions used are
    tanh/exp/copy -> one ACT table set ("exp_and_others"), loaded once.
  - h,c state kept doubled (H=2h, C=2c) so the LSTM update is exactly three
    fused scalar_tensor_tensor ops; the 0.5 factors fold into weights.
  - x-dependent parts of alpha/beta preacts fold into the SAME PSUM
    accumulation via 3 static rhs matrices against transposed x slices.
  - softmax: one 131-col exp (no ACT accumulator); row sums via DVE
    tensor_reduce; the alpha-sum and (s_a*s_b) product on the otherwise-idle
    Pool engine; 1/x on DVE.
  - Output head: Pool engine accumulates sum_t hT in SBUF; two matmuls at
    the very end project it with w_pred.
"""

import sys

import numpy as np
import ml_dtypes

for _p in ("/opt/trn_rl_repo",):
    if _p not in sys.path:
        sys.path.insert(0, _p)

import concourse.bass as bass
import concourse.bacc as bacc
import concourse.tile as tile
from concourse import mybir
from concourse.bass_utils import run_bass_kernel_spmd
from concourse.masks import make_identity

B, T, F, W, H, L = 1024, 256, 128, 3, 256, 1
NCORES = 8
BL = B // NCORES  # 128
AF = mybir.ActivationFunctionType
ALU = mybir.AluOpType
AX = mybir.AxisListType
DT = mybir.dt
F32 = DT.float32
BF16 = DT.bfloat16
NBF = ml_dtypes.bfloat16

_CACHE = {}


def build_kernel(gate_bias_nonzero, ab_bias_nonzero):
    nc = bacc.Bacc("TRN2")

    xn_d = nc.dram_tensor("xn", [T, BL, F], BF16, kind="ExternalInput")
    xt_d = nc.dram_tensor("xt", [T, F, BL], BF16, kind="ExternalInput")
    wg_d = nc.dram_tensor("wg", [3, 128, 1024], BF16, kind="ExternalInput")
    wab_d = nc.dram_tensor("wab", [2, 128, 131], BF16, kind="ExternalInput")
    wr_d = nc.dram_tensor("wr", [3, 128, 131], BF16, kind="ExternalInput")
    wp_d = nc.dram_tensor("wp", [128, 2], F32, kind="ExternalInput")
    bg_d = nc.dram_tensor("bg", [1, 1024], BF16, kind="ExternalInput")
    bab_d = nc.dram_tensor("bab", [1, 131], BF16, kind="ExternalInput")
    y_d = nc.dram_tensor("y", [1, BL], F32, kind="ExternalOutput")

    from contextlib import ExitStack

    with tile.TileContext(nc) as tc, ExitStack() as ctx:
        singles = ctx.enter_context(tc.tile_pool(name="singles", bufs=1))
        xn_pool = ctx.enter_context(tc.tile_pool(name="xn", bufs=3))
        xt_pool = ctx.enter_context(tc.tile_pool(name="xt", bufs=3))
        work = ctx.enter_context(tc.tile_pool(name="work", bufs=2))
        ab_pool = ctx.enter_context(tc.tile_pool(name="abps", bufs=3, space="PSUM"))
        g_pool = ctx.enter_context(tc.tile_pool(name="gps", bufs=1, space="PSUM"))
        tr_pool = ctx.enter_context(tc.tile_pool(name="trps", bufs=2, space="PSUM"))
        y_pool = ctx.enter_context(tc.tile_pool(name="yps", bufs=1, space="PSUM"))

        # ---- one-time loads -------------------------------------------------
        wg_s = singles.tile([128, 3, 1024], BF16)
        wab_s = singles.tile([128, 2, 131], BF16)
        wr_s = singles.tile([128, 3, 131], BF16)
        wp_s = singles.tile([128, 2], F32)
        for k in range(3):
            nc.sync.dma_start(out=wg_s[:, k, :], in_=wg_d[k])
            nc.sync.dma_start(out=wr_s[:, k, :], in_=wr_d[k])
        for k in range(2):
            nc.sync.dma_start(out=wab_s[:, k, :], in_=wab_d[k])
        nc.sync.dma_start(out=wp_s, in_=wp_d[:])
        ident = singles.tile([128, 128], BF16)
        make_identity(nc, ident)
        ones_row = None
        bg_s = bab_s = None
        if gate_bias_nonzero or ab_bias_nonzero:
            ones_row = singles.tile([1, 128], BF16)
            nc.vector.memset(ones_row, 1.0)
        if gate_bias_nonzero:
            bg_s = singles.tile([1, 1024], BF16)
            nc.sync.dma_start(out=bg_s, in_=bg_d[:])
        if ab_bias_nonzero:
            bab_s = singles.tile([1, 131], BF16)
            nc.sync.dma_start(out=bab_s, in_=bab_d[:])

        # ---- persistent state ----------------------------------------------
        H_s = singles.tile([128, 256], BF16)  # doubled hidden state 2*h
        C_s = singles.tile([128, 256], F32)   # doubled cell state 2*c
        Hsum = singles.tile([128, 256], F32)  # sum_t of transposed h (doubled)
        nc.gpsimd.memset(H_s, 0.0)
        nc.gpsimd.memset(C_s, 0.0)
        nc.gpsimd.memset(Hsum, 0.0)

        ab_tiles = {}
        xn_tiles = {}

        def new_ab(tau):
            ab_tiles[tau] = ab_pool.tile([128, 131], F32, tag="ab",
                                         name=f"ab{tau}")
            if ab_bias_nonzero:
                nc.tensor.matmul(
                    ab_tiles[tau], ones_row, bab_s, start=True, stop=False
                )

        # ab PSUM start flag: first writer per tile (see baseline comments).
        def ab_start(tau, first):
            return first and not ab_bias_nonzero

        GB = 4  # steps per DMA group
        xn_groups = {}
        xt_groups = {}
        for t in range(T):
            # -- DMA x slices in (batched, 4 steps per transfer) --------------
            if t % GB == 0:
                gi = t // GB
                gxn = xn_pool.tile([128, GB, 128], BF16, tag="xn",
                                   name=f"xng{gi}")
                nc.sync.dma_start(out=gxn,
                                  in_=xn_d[t:t + GB].rearrange("k p f -> p k f"))
                xn_groups[gi] = gxn
                gxt = xt_pool.tile([128, GB, 128], BF16, tag="xt",
                                   name=f"xtg{gi}")
                nc.sync.dma_start(out=gxt,
                                  in_=xt_d[t:t + GB].rearrange("k p f -> p k f"))
                xt_groups[gi] = gxt
            xn_tiles[t] = xn_groups[t // GB][:, t % GB, :]
            xt_t = xt_groups[t // GB][:, t % GB, :]

            # -- x contributions to alpha/beta preacts of steps t, t+1, t+2 ---
            if t == 0:
                for tau in range(min(3, T)):
                    new_ab(tau)
            elif t + 2 < T:
                new_ab(t + 2)
            # R2 -> psum[t], R1 -> psum[t+1], R0 -> psum[t+2]
            nc.tensor.matmul(
                ab_tiles[t], xt_t, wr_s[:, 2, :],
                start=ab_start(t, t == 0), stop=False,
            )
            if t + 1 < T:
                nc.tensor.matmul(
                    ab_tiles[t + 1], xt_t, wr_s[:, 1, :],
                    start=ab_start(t + 1, t == 0), stop=False,
                )
            if t + 2 < T:
                nc.tensor.matmul(
                    ab_tiles[t + 2], xt_t, wr_s[:, 0, :],
                    start=ab_start(t + 2, True), stop=False,
                )

            # gate-psum bias first-writers (general path; biases are zero in
            # this instance so this is skipped)
            g_ps = g_pool.tile([128, 1024], F32, tag="g")
            if gate_bias_nonzero:
                nc.tensor.matmul(g_ps[:, 0:512], ones_row, bg_s[:, 0:512],
                                 start=True, stop=False)
                nc.tensor.matmul(g_ps[:, 512:1024], ones_row,
                                 bg_s[:, 512:1024], start=True, stop=False)

            # -- transpose H (state from step t-1) ----------------------------
            tr_t = tr_pool.tile([128, 384], BF16, tag="tr")
            nc.tensor.transpose(tr_t[:, 0:128], H_s[:, 0:128], ident)
            nc.tensor.transpose(tr_t[:, 128:256], H_s[:, 128:256], ident)
            hT01 = work.tile([128, 256], BF16, tag="hT01")
            nc.vector.tensor_copy(out=hT01[:, 0:128], in_=tr_t[:, 0:128])
            nc.scalar.copy(out=hT01[:, 128:256], in_=tr_t[:, 128:256])
            hT0 = hT01[:, 0:128]
            hT1 = hT01[:, 128:256]

            # -- h-dependent matmuls: alpha/beta FIRST (critical path), then
            #    the gate h-parts which only gate the tail ---------------------
            nc.tensor.matmul(ab_tiles[t], hT0, wab_s[:, 0, :],
                             start=False, stop=False)
            nc.tensor.matmul(ab_tiles[t], hT1, wab_s[:, 1, :],
                             start=False, stop=True)
            gs = not gate_bias_nonzero
            nc.tensor.matmul(g_ps[:, 0:512], hT0, wg_s[:, 1, 0:512],
                             start=gs, stop=False)
            nc.tensor.matmul(g_ps[:, 512:1024], hT0, wg_s[:, 1, 512:1024],
                             start=gs, stop=False)
            nc.tensor.matmul(g_ps[:, 0:512], hT1, wg_s[:, 2, 0:512],
                             start=False, stop=False)
            nc.tensor.matmul(g_ps[:, 512:1024], hT1, wg_s[:, 2, 512:1024],
                             start=False, stop=False)

            # -- attention softmax path ---------------------------------------
            t_ab = work.tile([128, 131], F32, tag="tab")
            nc.scalar.activation(out=t_ab, in_=ab_tiles[t], func=AF.Tanh)
            e_ab = work.tile([128, 131], F32, tag="eab")
            nc.scalar.activation(out=e_ab, in_=t_ab, func=AF.Exp)

            # row sums: s_b on DVE; alpha-sum and product on Pool
            s_b = work.tile([128, 1], F32, tag="sb")
            nc.vector.tensor_reduce(out=s_b, in_=e_ab[:, 3:131],
                                    axis=AX.X, op=ALU.add)
            sa01 = work.tile([128, 1], F32, tag="sa01")
            nc.gpsimd.tensor_add(sa01, e_ab[:, 0:1], e_ab[:, 1:2])
            sa = work.tile([128, 1], F32, tag="sa")
            nc.gpsimd.tensor_add(sa, sa01, e_ab[:, 2:3])
            denom = work.tile([128, 1], F32, tag="den")
            nc.gpsimd.tensor_mul(denom, sa, s_b)

            # u = sum_w e_alpha[w] * x_{t-2+w}
            u = work.tile([128, 128], BF16, tag="u")
            if t == 0:
                nc.vector.tensor_scalar_mul(u, xn_tiles[0], e_ab[:, 2:3])
            elif t == 1:
                u01 = work.tile([128, 128], BF16, tag="u01")
                nc.vector.tensor_scalar_mul(u01, xn_tiles[0], e_ab[:, 1:2])
                nc.vector.scalar_tensor_tensor(
                    out=u, in0=xn_tiles[1], scalar=e_ab[:, 2:3], in1=u01,
                    op0=ALU.mult, op1=ALU.add)
            else:
                u01 = work.tile([128, 128], BF16, tag="u01")
                nc.vector.tensor_scalar_mul(u01, xn_tiles[t - 2], e_ab[:, 0:1])
                u02 = work.tile([128, 128], BF16, tag="u02")
                nc.vector.scalar_tensor_tensor(
                    out=u02, in0=xn_tiles[t - 1], scalar=e_ab[:, 1:2], in1=u01,
                    op0=ALU.mult, op1=ALU.add)
                nc.vector.scalar_tensor_tensor(
                    out=u, in0=xn_tiles[t], scalar=e_ab[:, 2:3], in1=u02,
                    op0=ALU.mult, op1=ALU.add)
            r_t = work.tile([128, 1], F32, tag="rt")
            nc.vector.reciprocal(r_t, denom)
            # z = e_beta * u * r  (normalized attention output)
            z = work.tile([128, 128], BF16, tag="z")
            nc.vector.scalar_tensor_tensor(
                out=z, in0=u, scalar=r_t, in1=e_ab[:, 3:131],
                op0=ALU.mult, op1=ALU.mult)

            # -- zT and gates-z -----------------------------------------------
            nc.tensor.transpose(tr_t[:, 256:384], z, ident)
            zT = work.tile([128, 128], BF16, tag="zT")
            nc.vector.tensor_copy(out=zT, in_=tr_t[:, 256:384])
            nc.tensor.matmul(g_ps[:, 0:512], zT, wg_s[:, 0, 0:512],
                             start=False, stop=True)
            nc.tensor.matmul(g_ps[:, 512:1024], zT, wg_s[:, 0, 512:1024],
                             start=False, stop=True)

            # -- gate activations: cols [f(256) i(256) o(256) g(256)] ---------
            tg4 = work.tile([128, 1024], BF16, tag="tg4")
            nc.scalar.activation(out=tg4[:, 0:512], in_=g_ps[:, 0:512],
                                 func=AF.Tanh, scale=0.5)
            nc.scalar.activation(out=tg4[:, 768:1024], in_=g_ps[:, 768:1024],
                                 func=AF.Tanh)
            nc.scalar.activation(out=tg4[:, 512:768], in_=g_ps[:, 512:768],
                                 func=AF.Tanh, scale=0.5)

            # -- LSTM state update (doubled state) ----------------------------
            # A = (1+tanh(f/2)) * C   (= 4*sig(f)*c)
            A_t = work.tile([128, 256], F32, tag="A")
            nc.vector.scalar_tensor_tensor(
                out=A_t, in0=tg4[:, 0:256], scalar=1.0, in1=C_s,
                op0=ALU.add, op1=ALU.mult)
            # Q = (1+tanh(i/2)) * tanh(g)   (= 2*sig(i)*tanh(g))
            Q_t = work.tile([128, 256], BF16, tag="Q")
            nc.vector.scalar_tensor_tensor(
                out=Q_t, in0=tg4[:, 256:512], scalar=1.0, in1=tg4[:, 768:1024],
                op0=ALU.add, op1=ALU.mult)
            # C_new = 0.5*A + Q   (= 2*c_new)
            nc.vector.scalar_tensor_tensor(
                out=C_s, in0=A_t, scalar=0.5, in1=Q_t,
                op0=ALU.mult, op1=ALU.add)
            # tanh(c_new) = tanh(0.5*C)
            t_c = work.tile([128, 256], BF16, tag="tc")
            nc.scalar.activation(out=t_c, in_=C_s, func=AF.Tanh, scale=0.5)
            # H_new = (1+tanh(o/2)) * tanh(c_new)   (= 2*h_new)
            nc.vector.scalar_tensor_tensor(
                out=H_s, in0=tg4[:, 512:768], scalar=1.0, in1=t_c,
                op0=ALU.add, op1=ALU.mult)

            # -- output-head accumulation on the idle Pool engine -------------
            if t > 0:  # h_0 == 0 contributes nothing
                nc.gpsimd.tensor_add(Hsum, Hsum, hT01)

        # ---- final h contribution + projection + writeback ------------------
        tr_f = tr_pool.tile([128, 384], BF16, tag="tr")
        nc.tensor.transpose(tr_f[:, 0:128], H_s[:, 0:128], ident)
        nc.tensor.transpose(tr_f[:, 128:256], H_s[:, 128:256], ident)
        hT01f = work.tile([128, 256], BF16, tag="hT01")
        nc.vector.tensor_copy(out=hT01f[:, 0:128], in_=tr_f[:, 0:128])
        nc.scalar.copy(out=hT01f[:, 128:256], in_=tr_f[:, 128:256])
        nc.gpsimd.tensor_add(Hsum, Hsum, hT01f)
        y_ps = y_pool.tile([1, 128], F32)
        nc.tensor.matmul(y_ps, wp_s[:, 0:1], Hsum[:, 0:128],
                         start=True, stop=False)
        nc.tensor.matmul(y_ps, wp_s[:, 1:2], Hsum[:, 128:256],
                         start=False, stop=True)
        y_sb = work.tile([1, 128], F32, tag="ysb")
        nc.scalar.copy(out=y_sb, in_=y_ps)
        nc.sync.dma_start(out=y_d[:], in_=y_sb)

    nc.finalize()
    return nc


def _prep_inputs(v, w_h_alpha, b_h_alpha, w_a_alpha, b_a_alpha,
                 w_h_beta, b_h_beta, w_a_beta, b_a_beta,
                 w_ih, b_ih, w_hh, b_hh, w_pred, b_pred):
    v = np.ascontiguousarray(np.asarray(v, dtype=np.float32))
    # gate row reorder: torch order (i,f,g,o) -> (f,i,o,g)
    idx = np.concatenate([np.arange(H, 2 * H), np.arange(0, H),
                          np.arange(3 * H, 4 * H), np.arange(2 * H, 3 * H)])
    wih_p = np.asarray(w_ih, np.float32)[idx]          # [1024, 128]
    whh_p = np.asarray(w_hh, np.float32)[idx]          # [1024, 256]
    bg = (np.asarray(b_ih, np.float32) + np.asarray(b_hh, np.float32))[idx]

    wg = np.zeros((3, 128, 1024), np.float32)
    wg[0] = wih_p.T
    wg[1] = 0.5 * whh_p.T[0:128]
    wg[2] = 0.5 * whh_p.T[128:256]

    wab = np.zeros((2, 128, 131), np.float32)
    wha_t = np.asarray(w_h_alpha, np.float32).T        # [H, 3]
    whb_t = np.asarray(w_h_beta, np.float32).T         # [H, F]
    for k in range(2):
        wab[k, :, 0:3] = 0.5 * wha_t[128 * k:128 * (k + 1)]
        wab[k, :, 3:131] = 0.5 * whb_t[128 * k:128 * (k + 1)]

    wr = np.zeros((3, 128, 131), np.float32)
    waa = np.asarray(w_a_alpha, np.float32)[0]         # [F]
    wab_beta = np.asarray(w_a_beta, np.float32)[0]     # [W]
    eye = np.eye(128, dtype=np.float32)
    for d in range(3):
        wr[d, :, d] = waa
        wr[d, :, 3:131] = wab_beta[d] * eye

    wp = np.zeros((128, 2), np.float32)
    wp[:, 0] = 0.5 * np.asarray(w_pred, np.float32)[0][0:128]
    wp[:, 1] = 0.5 * np.asarray(w_pred, np.float32)[0][128:256]

    bab = np.zeros((1, 131), np.float32)
    bab[0, 0:3] = np.asarray(b_h_alpha, np.float32) + np.asarray(b_a_alpha,
                                                                 np.float32)[0]
    bab[0, 3:131] = np.asarray(b_h_beta, np.float32) + np.asarray(b_a_beta,
                                                                  np.float32)[0]

    gate_bias_nonzero = bool(np.any(bg != 0.0))
    ab_bias_nonzero = bool(np.any(bab != 0.0))

    shared = {
        "wg": wg.astype(NBF), "wab": wab.astype(NBF), "wr": wr.astype(NBF),
        "wp": wp, "bg": bg.reshape(1, 1024).astype(NBF),
        "bab": bab.astype(NBF),
    }
    in_maps = []
    vb = v.astype(NBF)
    vs = vb.reshape(NCORES, BL, T, F)
    for c in range(NCORES):
        vc = vs[c]                                     # [BL, T, F]
        in_maps.append({
            "xn": np.ascontiguousarray(vc.transpose(1, 0, 2)),  # [T, BL, F]
            "xt": np.ascontiguousarray(vc.transpose(1, 2, 0)),  # [T, F, BL]
            **shared,
        })
    b_pred_total = float(T) * np.asarray(b_pred, np.float32)    # [L]
    return in_maps, gate_bias_nonzero, ab_bias_nonzero, b_pred_total


def _run(inputs, trace=False):
    in_maps, gb_nz, ab_nz, b_pred_total = _prep_inputs(**inputs)
    key = (gb_nz, ab_nz)
    if key not in _CACHE:
        _CACHE[key] = build_kernel(gb_nz, ab_nz)
    nc = _CACHE[key]
    res = run_bass_kernel_spmd(
        nc, in_maps, core_ids=list(range(NCORES)), trace=trace,
    )
    y = np.concatenate(
        [res.results[c]["y"].reshape(BL, L) for c in range(NCORES)], axis=0)
    y = y + b_pred_total[None, :]
    return np.asarray(y, dtype=np.float32), res


def kernel(**inputs):
    y, _ = _run(inputs, trace=False)
    return y


# revision 10
# speedup vs baseline: 1.3131x; 1.0157x over previous
"""Trainium2 Bass kernel for nn_BuildModel_3796751089773.

RAIM-attention + LSTMCell scan over T=256 steps, B=1024, F=128, H=256, W=3,
followed by sum-over-time prediction head -> [B, 1].

Strategy (8 cores, data-parallel over batch, B_local = 128 = SBUF partitions):
  - Normal layout [batch_partitions, feature_free] for attention softmax and
    all elementwise work (per-partition scalars make softmax/z cheap).
  - All matmuls/transposes in bf16 (1 cycle/row at any N, cheap LDWEIGHTS);
    PSUM accumulation stays fp32.  Cell state C kept fp32, everything else
    that feeds matmuls is bf16.
  - gates = z @ W_ih^T + h @ W_hh^T with activations-transposed as PE
    stationary (hT/zT via PE transposes), weights streaming as rhs.
  - sigmoid(x) = 0.5*(1+tanh(x/2)) so the only ACT functions used are
    tanh/exp/copy -> one ACT table set ("exp_and_others"), loaded once.
  - h,c state kept doubled (H=2h, C=2c) so the LSTM update is exactly three
    fused scalar_tensor_tensor ops; the 0.5 factors fold into weights.
  - x-dependent parts of alpha/beta preacts fold into the SAME PSUM
    accumulation via 3 static rhs matrices against transposed x slices.
  - softmax: one 131-col exp (no ACT accumulator); row sums via DVE
    tensor_reduce; the alpha-sum and (s_a*s_b) product on the otherwise-idle
    Pool engine; 1/x on DVE.
  - Output head: Pool engine accumulates sum_t hT in SBUF; two matmuls at
    the very end project it with w_pred.
"""

import sys

import numpy as np
import ml_dtypes

for _p in ("/opt/trn_rl_repo",):
    if _p not in sys.path:
        sys.path.insert(0, _p)

import concourse.bass as bass
import concourse.bacc as bacc
import concourse.tile as tile
from concourse import mybir
from concourse.bass_utils import run_bass_kernel_spmd
from concourse.masks import make_identity

B, T, F, W, H, L = 1024, 256, 128, 3, 256, 1
NCORES = 8
BL = B // NCORES  # 128
AF = mybir.ActivationFunctionType
ALU = mybir.AluOpType
AX = mybir.AxisListType
DT = mybir.dt
F32 = DT.float32
BF16 = DT.bfloat16
NBF = ml_dtypes.bfloat16

_CACHE = {}


def build_kernel(gate_bias_nonzero, ab_bias_nonzero):
    nc = bacc.Bacc("TRN2")

    xn_d = nc.dram_tensor("xn", [T, BL, F], BF16, kind="ExternalInput")
    xt_d = nc.dram_tensor("xt", [T, F, BL], BF16, kind="ExternalInput")
    wg_d = nc.dram_tensor("wg", [3, 128, 1024], BF16, kind="ExternalInput")
    wab_d = nc.dram_tensor("wab", [2, 128, 131], BF16, kind="ExternalInput")
    wr_d = nc.dram_tensor("wr", [3, 128, 131], BF16, kind="ExternalInput")
    wp_d = nc.dram_tensor("wp", [128, 2], F32, kind="ExternalInput")
    bg_d = nc.dram_tensor("bg", [1, 1024], BF16, kind="ExternalInput")
    bab_d = nc.dram_tensor("bab", [1, 131], BF16, kind="ExternalInput")
    y_d = nc.dram_tensor("y", [1, BL], F32, kind="ExternalOutput")

    from contextlib import ExitStack

    with tile.TileContext(nc) as tc, ExitStack() as ctx:
        singles = ctx.enter_context(tc.tile_pool(name="singles", bufs=1))
        xn_pool = ctx.enter_context(tc.tile_pool(name="xn", bufs=3))
        xt_pool = ctx.enter_context(tc.tile_pool(name="xt", bufs=3))
        work = ctx.enter_context(tc.tile_pool(name="work", bufs=2))
        hpool = ctx.enter_context(tc.tile_pool(name="hpool", bufs=4))
        ab_pool = ctx.enter_context(tc.tile_pool(name="abps", bufs=3, space="PSUM"))
        g_pool = ctx.enter_context(tc.tile_pool(name="gps", bufs=1, space="PSUM"))
        tr_pool = ctx.enter_context(tc.tile_pool(name="trps", bufs=2, space="PSUM"))
        y_pool = ctx.enter_context(tc.tile_pool(name="yps", bufs=1, space="PSUM"))

        # ---- one-time loads -------------------------------------------------
        wg_s = singles.tile([128, 3, 1024], BF16)
        wab_s = singles.tile([128, 2, 131], BF16)
        wr_s = singles.tile([128, 3, 131], BF16)
        wp_s = singles.tile([128, 2], F32)
        for k in range(3):
            nc.sync.dma_start(out=wg_s[:, k, :], in_=wg_d[k])
            nc.sync.dma_start(out=wr_s[:, k, :], in_=wr_d[k])
        for k in range(2):
            nc.sync.dma_start(out=wab_s[:, k, :], in_=wab_d[k])
        nc.sync.dma_start(out=wp_s, in_=wp_d[:])
        ident = singles.tile([128, 128], BF16)
        make_identity(nc, ident)
        ones_row = None
        bg_s = bab_s = None
        if gate_bias_nonzero or ab_bias_nonzero:
            ones_row = singles.tile([1, 128], BF16)
            nc.vector.memset(ones_row, 1.0)
        if gate_bias_nonzero:
            bg_s = singles.tile([1, 1024], BF16)
            nc.sync.dma_start(out=bg_s, in_=bg_d[:])
        if ab_bias_nonzero:
            bab_s = singles.tile([1, 131], BF16)
            nc.sync.dma_start(out=bab_s, in_=bab_d[:])

        # ---- persistent state ----------------------------------------------
        H_s = singles.tile([128, 256], BF16)  # doubled hidden state 2*h
        C_s = singles.tile([128, 256], F32)   # doubled cell state 2*c
        Hsum = singles.tile([128, 256], F32)  # sum_t of transposed h (doubled)
        nc.gpsimd.memset(H_s, 0.0)
        nc.gpsimd.memset(C_s, 0.0)
        nc.gpsimd.memset(Hsum, 0.0)

        ab_tiles = {}
        xn_tiles = {}

        def new_ab(tau):
            ab_tiles[tau] = ab_pool.tile([128, 131], F32, tag="ab",
                                         name=f"ab{tau}")
            if ab_bias_nonzero:
                nc.tensor.matmul(
                    ab_tiles[tau], ones_row, bab_s, start=True, stop=False
                )

        # ab PSUM start flag: first writer per tile (see baseline comments).
        def ab_start(tau, first):
            return first and not ab_bias_nonzero

        GB = 4  # steps per DMA group
        xn_groups = {}
        xt_groups = {}
        for t in range(T):
            # -- DMA x slices in (batched, 4 steps per transfer) --------------
            if t % GB == 0:
                gi = t // GB
                gxn = xn_pool.tile([128, GB, 128], BF16, tag="xn",
                                   name=f"xng{gi}")
                nc.sync.dma_start(out=gxn,
                                  in_=xn_d[t:t + GB].rearrange("k p f -> p k f"))
                xn_groups[gi] = gxn
                gxt = xt_pool.tile([128, GB, 128], BF16, tag="xt",
                                   name=f"xtg{gi}")
                nc.sync.dma_start(out=gxt,
                                  in_=xt_d[t:t + GB].rearrange("k p f -> p k f"))
                xt_groups[gi] = gxt
            xn_tiles[t] = xn_groups[t // GB][:, t % GB, :]
            xt_t = xt_groups[t // GB][:, t % GB, :]

            # -- x contributions to alpha/beta preacts of steps t, t+1, t+2 ---
            if t == 0:
                for tau in range(min(3, T)):
                    new_ab(tau)
            elif t + 2 < T:
                new_ab(t + 2)
            # R2 -> psum[t], R1 -> psum[t+1], R0 -> psum[t+2]
            nc.tensor.matmul(
                ab_tiles[t], xt_t, wr_s[:, 2, :],
                start=ab_start(t, t == 0), stop=False,
            )
            if t + 1 < T:
                nc.tensor.matmul(
                    ab_tiles[t + 1], xt_t, wr_s[:, 1, :],
                    start=ab_start(t + 1, t == 0), stop=False,
                )
            if t + 2 < T:
                nc.tensor.matmul(
                    ab_tiles[t + 2], xt_t, wr_s[:, 0, :],
                    start=ab_start(t + 2, True), stop=False,
                )

            # gate-psum bias first-writers (general path; biases are zero in
            # this instance so this is skipped)
            g_ps = g_pool.tile([128, 1024], F32, tag="g")
            if gate_bias_nonzero:
                nc.tensor.matmul(g_ps[:, 0:512], ones_row, bg_s[:, 0:512],
                                 start=True, stop=False)
                nc.tensor.matmul(g_ps[:, 512:1024], ones_row,
                                 bg_s[:, 512:1024], start=True, stop=False)

            # -- transpose H (state from step t-1) ----------------------------
            tr_t = tr_pool.tile([128, 384], BF16, tag="tr")
            nc.tensor.transpose(tr_t[:, 0:128], H_s[:, 0:128], ident)
            nc.tensor.transpose(tr_t[:, 128:256], H_s[:, 128:256], ident)
            hT01 = hpool.tile([128, 256], BF16, tag="hT01")
            nc.vector.tensor_copy(out=hT01[:, 0:128], in_=tr_t[:, 0:128])
            nc.vector.tensor_copy(out=hT01[:, 128:256], in_=tr_t[:, 128:256])
            hT0 = hT01[:, 0:128]
            hT1 = hT01[:, 128:256]
            # output-head accumulation on the idle Pool engine (early in the
            # Pool queue so its hT01 read never blocks a later step's copies)
            if t > 0:  # h_0 == 0 contributes nothing
                nc.gpsimd.tensor_add(Hsum, Hsum, hT01)

            # -- h-dependent matmuls: alpha/beta FIRST (critical path), then
            #    the gate h-parts which only gate the tail ---------------------
            nc.tensor.matmul(ab_tiles[t], hT0, wab_s[:, 0, :],
                             start=False, stop=False)
            nc.tensor.matmul(ab_tiles[t], hT1, wab_s[:, 1, :],
                             start=False, stop=True)
            gs = not gate_bias_nonzero
            nc.tensor.matmul(g_ps[:, 0:512], hT0, wg_s[:, 1, 0:512],
                             start=gs, stop=False)
            nc.tensor.matmul(g_ps[:, 512:1024], hT0, wg_s[:, 1, 512:1024],
                             start=gs, stop=False)
            nc.tensor.matmul(g_ps[:, 0:512], hT1, wg_s[:, 2, 0:512],
                             start=False, stop=False)
            nc.tensor.matmul(g_ps[:, 512:1024], hT1, wg_s[:, 2, 512:1024],
                             start=False, stop=False)

            # -- attention softmax path ---------------------------------------
            # exp_a first (unblocks the u-chain on DVE); exp_b accumulates
            # s_b in the ACT accumulator (read out by a separate scalar inst)
            t_ab = work.tile([128, 131], F32, tag="tab")
            nc.scalar.activation(out=t_ab, in_=ab_tiles[t], func=AF.Tanh)
            e_ab = work.tile([128, 131], F32, tag="eab")
            nc.scalar.activation(out=e_ab[:, 0:3], in_=t_ab[:, 0:3],
                                 func=AF.Exp)
            s_b = work.tile([128, 1], F32, tag="sb")
            nc.scalar.activation(out=e_ab[:, 3:131], in_=t_ab[:, 3:131],
                                 func=AF.Exp, accum_out=s_b)

            # alpha-sum and the (s_a*s_b) product on the Pool engine
            sa01 = work.tile([128, 1], F32, tag="sa01")
            nc.gpsimd.tensor_add(sa01, e_ab[:, 0:1], e_ab[:, 1:2])
            sa = work.tile([128, 1], F32, tag="sa")
            nc.gpsimd.tensor_add(sa, sa01, e_ab[:, 2:3])
            denom = work.tile([128, 1], F32, tag="den")
            nc.gpsimd.tensor_mul(denom, sa, s_b)

            # u = sum_w e_alpha[w] * x_{t-2+w}
            u = work.tile([128, 128], BF16, tag="u")
            if t == 0:
                nc.vector.tensor_scalar_mul(u, xn_tiles[0], e_ab[:, 2:3])
            elif t == 1:
                u01 = work.tile([128, 128], BF16, tag="u01")
                nc.vector.tensor_scalar_mul(u01, xn_tiles[0], e_ab[:, 1:2])
                nc.vector.scalar_tensor_tensor(
                    out=u, in0=xn_tiles[1], scalar=e_ab[:, 2:3], in1=u01,
                    op0=ALU.mult, op1=ALU.add)
            else:
                u01 = work.tile([128, 128], BF16, tag="u01")
                nc.vector.tensor_scalar_mul(u01, xn_tiles[t - 2], e_ab[:, 0:1])
                u02 = work.tile([128, 128], BF16, tag="u02")
                nc.vector.scalar_tensor_tensor(
                    out=u02, in0=xn_tiles[t - 1], scalar=e_ab[:, 1:2], in1=u01,
                    op0=ALU.mult, op1=ALU.add)
                nc.vector.scalar_tensor_tensor(
                    out=u, in0=xn_tiles[t], scalar=e_ab[:, 2:3], in1=u02,
                    op0=ALU.mult, op1=ALU.add)
            r_t = work.tile([128, 1], F32, tag="rt")
            nc.vector.reciprocal(r_t, denom)
            # z = e_beta * u * r  (normalized attention output)
            z = work.tile([128, 128], BF16, tag="z")
            nc.vector.scalar_tensor_tensor(
                out=z, in0=u, scalar=r_t, in1=e_ab[:, 3:131],
                op0=ALU.mult, op1=ALU.mult)

            # -- zT and gates-z -----------------------------------------------
            nc.tensor.transpose(tr_t[:, 256:384], z, ident)
            zT = work.tile([128, 128], BF16, tag="zT")
            nc.vector.tensor_copy(out=zT, in_=tr_t[:, 256:384])
            nc.tensor.matmul(g_ps[:, 0:512], zT, wg_s[:, 0, 0:512],
                             start=False, stop=True)
            nc.tensor.matmul(g_ps[:, 512:1024], zT, wg_s[:, 0, 512:1024],
                             start=False, stop=True)

            # -- gate activations: cols [f(256) i(256) o(256) g(256)] ---------
            tg4 = work.tile([128, 1024], BF16, tag="tg4")
            nc.scalar.activation(out=tg4[:, 0:512], in_=g_ps[:, 0:512],
                                 func=AF.Tanh, scale=0.5)
            nc.scalar.activation(out=tg4[:, 768:1024], in_=g_ps[:, 768:1024],
                                 func=AF.Tanh)
            nc.scalar.activation(out=tg4[:, 512:768], in_=g_ps[:, 512:768],
                                 func=AF.Tanh, scale=0.5)

            # -- LSTM state update (doubled state) ----------------------------
            # A = (1+tanh(f/2)) * C   (= 4*sig(f)*c)
            A_t = work.tile([128, 256], F32, tag="A")
            nc.vector.scalar_tensor_tensor(
                out=A_t, in0=tg4[:, 0:256], scalar=1.0, in1=C_s,
                op0=ALU.add, op1=ALU.mult)
            # Q = (1+tanh(i/2)) * tanh(g)   (= 2*sig(i)*tanh(g))
            Q_t = work.tile([128, 256], BF16, tag="Q")
            nc.vector.scalar_tensor_tensor(
                out=Q_t, in0=tg4[:, 256:512], scalar=1.0, in1=tg4[:, 768:1024],
                op0=ALU.add, op1=ALU.mult)
            # C_new = 0.5*A + Q   (= 2*c_new)
            nc.vector.scalar_tensor_tensor(
                out=C_s, in0=A_t, scalar=0.5, in1=Q_t,
                op0=ALU.mult, op1=ALU.add)
            # tanh(c_new) = tanh(0.5*C)
            t_c = work.tile([128, 256], BF16, tag="tc")
            nc.scalar.activation(out=t_c, in_=C_s, func=AF.Tanh, scale=0.5)
            # H_new = (1+tanh(o/2)) * tanh(c_new)   (= 2*h_new)
            nc.vector.scalar_tensor_tensor(
                out=H_s, in0=tg4[:, 512:768], scalar=1.0, in1=t_c,
                op0=ALU.add, op1=ALU.mult)

        # ---- final h contribution + projection + writeback ------------------
        tr_f = tr_pool.tile([128, 384], BF16, tag="tr")
        nc.tensor.transpose(tr_f[:, 0:128], H_s[:, 0:128], ident)
        nc.tensor.transpose(tr_f[:, 128:256], H_s[:, 128:256], ident)
        hT01f = hpool.tile([128, 256], BF16, tag="hT01")
        nc.vector.tensor_copy(out=hT01f[:, 0:128], in_=tr_f[:, 0:128])
        nc.vector.tensor_copy(out=hT01f[:, 128:256], in_=tr_f[:, 128:256])
        nc.gpsimd.tensor_add(Hsum, Hsum, hT01f)
        y_ps = y_pool.tile([1, 128], F32)
        nc.tensor.matmul(y_ps, wp_s[:, 0:1], Hsum[:, 0:128],
                         start=True, stop=False)
        nc.tensor.matmul(y_ps, wp_s[:, 1:2], Hsum[:, 128:256],
                         start=False, stop=True)
        y_sb = work.tile([1, 128], F32, tag="ysb")
        nc.scalar.copy(out=y_sb, in_=y_ps)
        nc.sync.dma_start(out=y_d[:], in_=y_sb)

    nc.finalize()
    return nc


def _prep_inputs(v, w_h_alpha, b_h_alpha, w_a_alpha, b_a_alpha,
                 w_h_beta, b_h_beta, w_a_beta, b_a_beta,
                 w_ih, b_ih, w_hh, b_hh, w_pred, b_pred):
    v = np.ascontiguousarray(np.asarray(v, dtype=np.float32))
    # gate row reorder: torch order (i,f,g,o) -> (f,i,o,g)
    idx = np.concatenate([np.arange(H, 2 * H), np.arange(0, H),
                          np.arange(3 * H, 4 * H), np.arange(2 * H, 3 * H)])
    wih_p = np.asarray(w_ih, np.float32)[idx]          # [1024, 128]
    whh_p = np.asarray(w_hh, np.float32)[idx]          # [1024, 256]
    bg = (np.asarray(b_ih, np.float32) + np.asarray(b_hh, np.float32))[idx]

    wg = np.zeros((3, 128, 1024), np.float32)
    wg[0] = wih_p.T
    wg[1] = 0.5 * whh_p.T[0:128]
    wg[2] = 0.5 * whh_p.T[128:256]

    wab = np.zeros((2, 128, 131), np.float32)
    wha_t = np.asarray(w_h_alpha, np.float32).T        # [H, 3]
    whb_t = np.asarray(w_h_beta, np.float32).T         # [H, F]
    for k in range(2):
        wab[k, :, 0:3] = 0.5 * wha_t[128 * k:128 * (k + 1)]
        wab[k, :, 3:131] = 0.5 * whb_t[128 * k:128 * (k + 1)]

    wr = np.zeros((3, 128, 131), np.float32)
    waa = np.asarray(w_a_alpha, np.float32)[0]         # [F]
    wab_beta = np.asarray(w_a_beta, np.float32)[0]     # [W]
    eye = np.eye(128, dtype=np.float32)
    for d in range(3):
        wr[d, :, d] = waa
        wr[d, :, 3:131] = wab_beta[d] * eye

    wp = np.zeros((128, 2), np.float32)
    wp[:, 0] = 0.5 * np.asarray(w_pred, np.float32)[0][0:128]
    wp[:, 1] = 0.5 * np.asarray(w_pred, np.float32)[0][128:256]

    bab = np.zeros((1, 131), np.float32)
    bab[0, 0:3] = np.asarray(b_h_alpha, np.float32) + np.asarray(b_a_alpha,
                                                                 np.float32)[0]
    bab[0, 3:131] = np.asarray(b_h_beta, np.float32) + np.asarray(b_a_beta,
                                                                  np.float32)[0]

    gate_bias_nonzero = bool(np.any(bg != 0.0))
    ab_bias_nonzero = bool(np.any(bab != 0.0))

    shared = {
        "wg": wg.astype(NBF), "wab": wab.astype(NBF), "wr": wr.astype(NBF),
        "wp": wp, "bg": bg.reshape(1, 1024).astype(NBF),
        "bab": bab.astype(NBF),
    }
    in_maps = []
    vb = v.astype(NBF)
    vs = vb.reshape(NCORES, BL, T, F)
    for c in range(NCORES):
        vc = vs[c]                                     # [BL, T, F]
        in_maps.append({
            "xn": np.ascontiguousarray(vc.transpose(1, 0, 2)),  # [T, BL, F]
            "xt": np.ascontiguousarray(vc.transpose(1, 2, 0)),  # [T, F, BL]
            **shared,
        })
    b_pred_total = float(T) * np.asarray(b_pred, np.float32)    # [L]
    return in_maps, gate_bias_nonzero, ab_bias_nonzero, b_pred_total


def _run(inputs, trace=False):
    in_maps, gb_nz, ab_nz, b_pred_total = _prep_inputs(**inputs)
    key = (gb_nz, ab_nz)
    if key not in _CACHE:
        _CACHE[key] = build_kernel(gb_nz, ab_nz)
    nc = _CACHE[key]
    res = run_bass_kernel_spmd(
        nc, in_maps, core_ids=list(range(NCORES)), trace=trace,
    )
    y = np.concatenate(
        [res.results[c]["y"].reshape(BL, L) for c in range(NCORES)], axis=0)
    y = y + b_pred_total[None, :]
    return np.asarray(y, dtype=np.float32), res


def kernel(**inputs):
    y, _ = _run(inputs, trace=False)
    return y


# revision 14
# speedup vs baseline: 1.5354x; 1.1693x over previous
"""Trainium2 Bass kernel for nn_BuildModel_3796751089773.

RAIM-attention + LSTMCell scan over T=256 steps, B=1024, F=128, H=256, W=3,
followed by sum-over-time prediction head -> [B, 1].

Strategy (8 cores, data-parallel over batch, B_local = 128 = SBUF partitions):
  - Normal layout [batch_partitions, feature_free] for attention softmax and
    all elementwise work (per-partition scalars make softmax/z cheap).
  - All matmuls/transposes in bf16 (1 cycle/row at any N, cheap LDWEIGHTS);
    PSUM accumulation stays fp32.  Cell state C kept fp32, everything else
    that feeds matmuls is bf16.
  - gates = z @ W_ih^T + h @ W_hh^T with activations-transposed as PE
    stationary (hT/zT via PE transposes), weights streaming as rhs.
  - sigmoid(x) = 0.5*(1+tanh(x/2)) so the only ACT functions used are
    tanh/exp/copy -> one ACT table set ("exp_and_others"), loaded once.
  - h,c state kept doubled (H=2h, C=2c) so the LSTM update is exactly three
    fused scalar_tensor_tensor ops; the 0.5 factors fold into weights.
  - x-dependent parts of alpha/beta preacts fold into the SAME PSUM
    accumulation via 3 static rhs matrices against transposed x slices.
  - softmax: one 131-col exp (no ACT accumulator); row sums via DVE
    tensor_reduce; the alpha-sum and (s_a*s_b) product on the otherwise-idle
    Pool engine; 1/x on DVE.
  - Output head: Pool engine accumulates sum_t hT in SBUF; two matmuls at
    the very end project it with w_pred.
"""

import sys

import numpy as np
import ml_dtypes

for _p in ("/opt/trn_rl_repo",):
    if _p not in sys.path:
        sys.path.insert(0, _p)

import concourse.bass as bass
import concourse.bacc as bacc
import concourse.tile as tile
from concourse import mybir
from concourse.bass_utils import run_bass_kernel_spmd
from concourse.masks import make_identity

B, T, F, W, H, L = 1024, 256, 128, 3, 256, 1
NCORES = 8
BL = B // NCORES  # 128
AF = mybir.ActivationFunctionType
ALU = mybir.AluOpType
AX = mybir.AxisListType
DT = mybir.dt
F32 = DT.float32
BF16 = DT.bfloat16
NBF = ml_dtypes.bfloat16

_CACHE = {}


def build_kernel(gate_bias_nonzero, ab_bias_nonzero):
    nc = bacc.Bacc("TRN2")

    xn_d = nc.dram_tensor("xn", [T, BL, F], BF16, kind="ExternalInput")
    xt_d = nc.dram_tensor("xt", [T, F, BL], BF16, kind="ExternalInput")
    wg_d = nc.dram_tensor("wg", [3, 128, 1024], BF16, kind="ExternalInput")
    wab_d = nc.dram_tensor("wab", [2, 128, 131], BF16, kind="ExternalInput")
    wr_d = nc.dram_tensor("wr", [3, 128, 131], BF16, kind="ExternalInput")
    wp_d = nc.dram_tensor("wp", [128, 2], F32, kind="ExternalInput")
    bg_d = nc.dram_tensor("bg", [1, 1024], BF16, kind="ExternalInput")
    bab_d = nc.dram_tensor("bab", [1, 131], BF16, kind="ExternalInput")
    y_d = nc.dram_tensor("y", [1, BL], F32, kind="ExternalOutput")

    from contextlib import ExitStack

    with tile.TileContext(nc) as tc, ExitStack() as ctx:
        singles = ctx.enter_context(tc.tile_pool(name="singles", bufs=1))
        xn_pool = ctx.enter_context(tc.tile_pool(name="xn", bufs=3))
        xt_pool = ctx.enter_context(tc.tile_pool(name="xt", bufs=3))
        work = ctx.enter_context(tc.tile_pool(name="work", bufs=2))
        hpool = ctx.enter_context(tc.tile_pool(name="hpool", bufs=4))
        ab_pool = ctx.enter_context(tc.tile_pool(name="abps", bufs=3, space="PSUM"))
        g_pool = ctx.enter_context(tc.tile_pool(name="gps", bufs=1, space="PSUM"))
        tr_pool = ctx.enter_context(tc.tile_pool(name="trps", bufs=2, space="PSUM"))
        y_pool = ctx.enter_context(tc.tile_pool(name="yps", bufs=1, space="PSUM"))

        # ---- one-time loads -------------------------------------------------
        wg_s = singles.tile([128, 3, 1024], BF16)
        wab_s = singles.tile([128, 2, 131], BF16)
        wr_s = singles.tile([128, 3, 131], BF16)
        wp_s = singles.tile([128, 2], F32)
        for k in range(3):
            nc.sync.dma_start(out=wg_s[:, k, :], in_=wg_d[k])
            nc.sync.dma_start(out=wr_s[:, k, :], in_=wr_d[k])
        for k in range(2):
            nc.sync.dma_start(out=wab_s[:, k, :], in_=wab_d[k])
        nc.sync.dma_start(out=wp_s, in_=wp_d[:])
        ident = singles.tile([128, 128], BF16)
        make_identity(nc, ident)
        ones_row = None
        bg_s = bab_s = None
        if gate_bias_nonzero or ab_bias_nonzero:
            ones_row = singles.tile([1, 128], BF16)
            nc.vector.memset(ones_row, 1.0)
        if gate_bias_nonzero:
            bg_s = singles.tile([1, 1024], BF16)
            nc.sync.dma_start(out=bg_s, in_=bg_d[:])
        if ab_bias_nonzero:
            bab_s = singles.tile([1, 131], BF16)
            nc.sync.dma_start(out=bab_s, in_=bab_d[:])

        # ---- persistent state ----------------------------------------------
        H_s = singles.tile([128, 256], BF16)  # doubled hidden state 2*h
        C_s = singles.tile([128, 256], F32)   # doubled cell state 2*c
        Hsum = singles.tile([128, 256], F32)  # sum_t of transposed h (doubled)
        nc.gpsimd.memset(H_s, 0.0)
        nc.gpsimd.memset(C_s, 0.0)
        nc.gpsimd.memset(Hsum, 0.0)

        ab_tiles = {}
        xn_tiles = {}

        def new_ab(tau):
            ab_tiles[tau] = ab_pool.tile([128, 131], F32, tag="ab",
                                         name=f"ab{tau}")
            if ab_bias_nonzero:
                nc.tensor.matmul(
                    ab_tiles[tau], ones_row, bab_s, start=True, stop=False
                )

        # ab PSUM start flag: first writer per tile (see baseline comments).
        def ab_start(tau, first):
            return first and not ab_bias_nonzero

        GB = 4  # steps per DMA group
        xn_groups = {}
        xt_groups = {}
        for t in range(T):
            # -- DMA x slices in (batched, 4 steps per transfer) --------------
            if t % GB == 0:
                gi = t // GB
                gxn = xn_pool.tile([128, GB, 128], BF16, tag="xn",
                                   name=f"xng{gi}")
                nc.sync.dma_start(out=gxn,
                                  in_=xn_d[t:t + GB].rearrange("k p f -> p k f"))
                xn_groups[gi] = gxn
                gxt = xt_pool.tile([128, GB, 128], BF16, tag="xt",
                                   name=f"xtg{gi}")
                nc.sync.dma_start(out=gxt,
                                  in_=xt_d[t:t + GB].rearrange("k p f -> p k f"))
                xt_groups[gi] = gxt
            xn_tiles[t] = xn_groups[t // GB][:, t % GB, :]
            xt_t = xt_groups[t // GB][:, t % GB, :]

            # -- x contributions to alpha/beta preacts of steps t, t+1, t+2 ---
            if t == 0:
                for tau in range(min(3, T)):
                    new_ab(tau)
            elif t + 2 < T:
                new_ab(t + 2)
            # R2 -> psum[t], R1 -> psum[t+1], R0 -> psum[t+2]
            nc.tensor.matmul(
                ab_tiles[t], xt_t, wr_s[:, 2, :],
                start=ab_start(t, t == 0), stop=False,
            )
            if t + 1 < T:
                nc.tensor.matmul(
                    ab_tiles[t + 1], xt_t, wr_s[:, 1, :],
                    start=ab_start(t + 1, t == 0), stop=False,
                )
            if t + 2 < T:
                nc.tensor.matmul(
                    ab_tiles[t + 2], xt_t, wr_s[:, 0, :],
                    start=ab_start(t + 2, True), stop=False,
                )

            # gate-psum bias first-writers (general path; biases are zero in
            # this instance so this is skipped).  The f,i half and the o,g
            # half live in separate PSUM tiles so the fi-tanh only waits on
            # its own half's final matmul.
            g_fi = g_pool.tile([128, 512], F32, tag="gfi")
            g_og = g_pool.tile([128, 512], F32, tag="gog")
            if gate_bias_nonzero:
                nc.tensor.matmul(g_fi, ones_row, bg_s[:, 0:512],
                                 start=True, stop=False)
                nc.tensor.matmul(g_og, ones_row,
                                 bg_s[:, 512:1024], start=True, stop=False)

            # -- transpose H (state from step t-1) ----------------------------
            tr_t = tr_pool.tile([128, 384], BF16, tag="tr")
            nc.tensor.transpose(tr_t[:, 0:128], H_s[:, 0:128], ident)
            nc.tensor.transpose(tr_t[:, 128:256], H_s[:, 128:256], ident)
            hT01 = hpool.tile([128, 256], BF16, tag="hT01")
            nc.vector.tensor_copy(out=hT01, in_=tr_t[:, 0:256])
            hT0 = hT01[:, 0:128]
            hT1 = hT01[:, 128:256]
            # output-head accumulation on the idle Pool engine (early in the
            # Pool queue so its hT01 read never blocks a later step's copies)
            if t > 0:  # h_0 == 0 contributes nothing
                nc.gpsimd.tensor_add(Hsum, Hsum, hT01)

            # -- h-dependent matmuls: alpha/beta FIRST (critical path), then
            #    the gate h-parts which only gate the tail ---------------------
            nc.tensor.matmul(ab_tiles[t], hT0, wab_s[:, 0, :],
                             start=False, stop=False)
            nc.tensor.matmul(ab_tiles[t], hT1, wab_s[:, 1, :],
                             start=False, stop=True)
            gs = not gate_bias_nonzero
            nc.tensor.matmul(g_fi, hT0, wg_s[:, 1, 0:512],
                             start=gs, stop=False)
            nc.tensor.matmul(g_og, hT0, wg_s[:, 1, 512:1024],
                             start=gs, stop=False)
            nc.tensor.matmul(g_fi, hT1, wg_s[:, 2, 0:512],
                             start=False, stop=False)
            nc.tensor.matmul(g_og, hT1, wg_s[:, 2, 512:1024],
                             start=False, stop=False)

            # -- attention softmax path ---------------------------------------
            # exp_a first (unblocks the u-chain on DVE); exp_b accumulates
            # s_b in the ACT accumulator (read out by a separate scalar inst)
            t_ab = work.tile([128, 131], F32, tag="tab")
            nc.scalar.activation(out=t_ab, in_=ab_tiles[t], func=AF.Tanh)
            e_ab = work.tile([128, 131], F32, tag="eab")
            nc.scalar.activation(out=e_ab[:, 0:3], in_=t_ab[:, 0:3],
                                 func=AF.Exp)
            s_b = work.tile([128, 1], F32, tag="sb")
            nc.scalar.activation(out=e_ab[:, 3:131], in_=t_ab[:, 3:131],
                                 func=AF.Exp, accum_out=s_b)

            # alpha-sum and the (s_a*s_b) product on the Pool engine
            sa01 = work.tile([128, 1], F32, tag="sa01")
            nc.gpsimd.tensor_add(sa01, e_ab[:, 0:1], e_ab[:, 1:2])
            sa = work.tile([128, 1], F32, tag="sa")
            nc.gpsimd.tensor_add(sa, sa01, e_ab[:, 2:3])
            denom = work.tile([128, 1], F32, tag="den")
            nc.gpsimd.tensor_mul(denom, sa, s_b)

            # u = sum_w e_alpha[w] * x_{t-2+w}
            u = work.tile([128, 128], BF16, tag="u")
            if t == 0:
                nc.vector.tensor_scalar_mul(u, xn_tiles[0], e_ab[:, 2:3])
            elif t == 1:
                u01 = work.tile([128, 128], BF16, tag="u01")
                nc.vector.tensor_scalar_mul(u01, xn_tiles[0], e_ab[:, 1:2])
                nc.vector.scalar_tensor_tensor(
                    out=u, in0=xn_tiles[1], scalar=e_ab[:, 2:3], in1=u01,
                    op0=ALU.mult, op1=ALU.add)
            else:
                u01 = work.tile([128, 128], BF16, tag="u01")
                nc.vector.tensor_scalar_mul(u01, xn_tiles[t - 2], e_ab[:, 0:1])
                u02 = work.tile([128, 128], BF16, tag="u02")
                nc.vector.scalar_tensor_tensor(
                    out=u02, in0=xn_tiles[t - 1], scalar=e_ab[:, 1:2], in1=u01,
                    op0=ALU.mult, op1=ALU.add)
                nc.vector.scalar_tensor_tensor(
                    out=u, in0=xn_tiles[t], scalar=e_ab[:, 2:3], in1=u02,
                    op0=ALU.mult, op1=ALU.add)
            r_t = work.tile([128, 1], F32, tag="rt")
            nc.vector.reciprocal(r_t, denom)
            # z = e_beta * u * r  (normalized attention output)
            z = work.tile([128, 128], BF16, tag="z")
            nc.vector.scalar_tensor_tensor(
                out=z, in0=u, scalar=r_t, in1=e_ab[:, 3:131],
                op0=ALU.mult, op1=ALU.mult)

            # -- zT and gates-z -----------------------------------------------
            nc.tensor.transpose(tr_t[:, 256:384], z, ident)
            zT = work.tile([128, 128], BF16, tag="zT")
            nc.vector.tensor_copy(out=zT, in_=tr_t[:, 256:384])
            nc.tensor.matmul(g_fi, zT, wg_s[:, 0, 0:512],
                             start=False, stop=True)
            nc.tensor.matmul(g_og, zT, wg_s[:, 0, 512:1024],
                             start=False, stop=True)

            # -- gate activations: cols [f(256) i(256) o(256) g(256)] ---------
            tg4 = work.tile([128, 1024], BF16, tag="tg4")
            nc.scalar.activation(out=tg4[:, 0:512], in_=g_fi,
                                 func=AF.Tanh, scale=0.5)
            nc.scalar.activation(out=tg4[:, 768:1024], in_=g_og[:, 256:512],
                                 func=AF.Tanh)
            nc.scalar.activation(out=tg4[:, 512:768], in_=g_og[:, 0:256],
                                 func=AF.Tanh, scale=0.5)

            # -- LSTM state update (doubled state) ----------------------------
            # A = (1+tanh(f/2)) * C   (= 4*sig(f)*c)
            A_t = work.tile([128, 256], F32, tag="A")
            nc.vector.scalar_tensor_tensor(
                out=A_t, in0=tg4[:, 0:256], scalar=1.0, in1=C_s,
                op0=ALU.add, op1=ALU.mult)
            # Q = (1+tanh(i/2)) * tanh(g)   (= 2*sig(i)*tanh(g))
            Q_t = work.tile([128, 256], BF16, tag="Q")
            nc.vector.scalar_tensor_tensor(
                out=Q_t, in0=tg4[:, 256:512], scalar=1.0, in1=tg4[:, 768:1024],
                op0=ALU.add, op1=ALU.mult)
            # C_new = 0.5*A + Q   (= 2*c_new)
            nc.vector.scalar_tensor_tensor(
                out=C_s, in0=A_t, scalar=0.5, in1=Q_t,
                op0=ALU.mult, op1=ALU.add)
            # tanh(c_new) = tanh(0.5*C)
            t_c = work.tile([128, 256], BF16, tag="tc")
            nc.scalar.activation(out=t_c, in_=C_s, func=AF.Tanh, scale=0.5)
            # H_new = (1+tanh(o/2)) * tanh(c_new)   (= 2*h_new)
            nc.vector.scalar_tensor_tensor(
                out=H_s, in0=tg4[:, 512:768], scalar=1.0, in1=t_c,
                op0=ALU.add, op1=ALU.mult)

        # ---- final h contribution + projection + writeback ------------------
        tr_f = tr_pool.tile([128, 384], BF16, tag="tr")
        nc.tensor.transpose(tr_f[:, 0:128], H_s[:, 0:128], ident)
        nc.tensor.transpose(tr_f[:, 128:256], H_s[:, 128:256], ident)
        hT01f = hpool.tile([128, 256], BF16, tag="hT01")
        nc.vector.tensor_copy(out=hT01f[:, 0:128], in_=tr_f[:, 0:128])
        nc.vector.tensor_copy(out=hT01f[:, 128:256], in_=tr_f[:, 128:256])
        nc.gpsimd.tensor_add(Hsum, Hsum, hT01f)
        y_ps = y_pool.tile([1, 128], F32)
        nc.tensor.matmul(y_ps, wp_s[:, 0:1], Hsum[:, 0:128],
                         start=True, stop=False)
        nc.tensor.matmul(y_ps, wp_s[:, 1:2], Hsum[:, 128:256],
                         start=False, stop=True)
        y_sb = work.tile([1, 128], F32, tag="ysb")
        nc.scalar.copy(out=y_sb, in_=y_ps)
        nc.sync.dma_start(out=y_d[:], in_=y_sb)

    nc.finalize()
    return nc


def _prep_inputs(v, w_h_alpha, b_h_alpha, w_a_alpha, b_a_alpha,
                 w_h_beta, b_h_beta, w_a_beta, b_a_beta,
                 w_ih, b_ih, w_hh, b_hh, w_pred, b_pred):
    v = np.ascontiguousarray(np.asarray(v, dtype=np.float32))
    # gate row reorder: torch order (i,f,g,o) -> (f,i,o,g)
    idx = np.concatenate([np.arange(H, 2 * H), np.arange(0, H),
                          np.arange(3 * H, 4 * H), np.arange(2 * H, 3 * H)])
    wih_p = np.asarray(w_ih, np.float32)[idx]          # [1024, 128]
    whh_p = np.asarray(w_hh, np.float32)[idx]          # [1024, 256]
    bg = (np.asarray(b_ih, np.float32) + np.asarray(b_hh, np.float32))[idx]

    wg = np.zeros((3, 128, 1024), np.float32)
    wg[0] = wih_p.T
    wg[1] = 0.5 * whh_p.T[0:128]
    wg[2] = 0.5 * whh_p.T[128:256]

    wab = np.zeros((2, 128, 131), np.float32)
    wha_t = np.asarray(w_h_alpha, np.float32).T        # [H, 3]
    whb_t = np.asarray(w_h_beta, np.float32).T         # [H, F]
    for k in range(2):
        wab[k, :, 0:3] = 0.5 * wha_t[128 * k:128 * (k + 1)]
        wab[k, :, 3:131] = 0.5 * whb_t[128 * k:128 * (k + 1)]

    wr = np.zeros((3, 128, 131), np.float32)
    waa = np.asarray(w_a_alpha, np.float32)[0]         # [F]
    wab_beta = np.asarray(w_a_beta, np.float32)[0]     # [W]
    eye = np.eye(128, dtype=np.float32)
    for d in range(3):
        wr[d, :, d] = waa
        wr[d, :, 3:131] = wab_beta[d] * eye

    wp = np.zeros((128, 2), np.float32)
    wp[:, 0] = 0.5 * np.asarray(w_pred, np.float32)[0][0:128]
    wp[:, 1] = 0.5 * np.asarray(w_pred, np.float32)[0][128:256]

    bab = np.zeros((1, 131), np.float32)
    bab[0, 0:3] = np.asarray(b_h_alpha, np.float32) + np.asarray(b_a_alpha,
                                                                 np.float32)[0]
    bab[0, 3:131] = np.asarray(b_h_beta, np.float32) + np.asarray(b_a_beta,
                                                                  np.float32)[0]

    gate_bias_nonzero = bool(np.any(bg != 0.0))
    ab_bias_nonzero = bool(np.any(bab != 0.0))

    shared = {
        "wg": wg.astype(NBF), "wab": wab.astype(NBF), "wr": wr.astype(NBF),
        "wp": wp, "bg": bg.reshape(1, 1024).astype(NBF),
        "bab": bab.astype(NBF),
    }
    in_maps = []
    vb = v.astype(NBF)
    vs = vb.reshape(NCORES, BL, T, F)
    for c in range(NCORES):
        vc = vs[c]                                     # [BL, T, F]
        in_maps.append({
            "xn": np.ascontiguousarray(vc.transpose(1, 0, 2)),  # [T, BL, F]
            "xt": np.ascontiguousarray(vc.transpose(1, 2, 0)),  # [T, F, BL]
            **shared,
        })
    b_pred_total = float(T) * np.asarray(b_pred, np.float32)    # [L]
    return in_maps, gate_bias_nonzero, ab_bias_nonzero, b_pred_total


def _run(inputs, trace=False):
    in_maps, gb_nz, ab_nz, b_pred_total = _prep_inputs(**inputs)
    key = (gb_nz, ab_nz)
    if key not in _CACHE:
        _CACHE[key] = build_kernel(gb_nz, ab_nz)
    nc = _CACHE[key]
    res = run_bass_kernel_spmd(
        nc, in_maps, core_ids=list(range(NCORES)), trace=trace,
    )
    y = np.concatenate(
        [res.results[c]["y"].reshape(BL, L) for c in range(NCORES)], axis=0)
    y = y + b_pred_total[None, :]
    return np.asarray(y, dtype=np.float32), res


def kernel(**inputs):
    y, _ = _run(inputs, trace=False)
    return y
